# revision 1
# baseline (speedup 1.0000x reference)
"""Trainium2 Bass kernel for the CAP loss (camera-aware proxy memory bank).

Strategy (8 NeuronCores, SPMD, raw Bass engine blocks):
  - The center bank [32000, 2048] is sharded along the center axis: 4000
    centers (= 500 labels x 8 cams, label-major) per core, pre-transposed and
    cast to bf16 on the host so each core streams a [2048, 4000] bf16 shard
    as 8 fully-contiguous 2MB slabs.
  - feats are replicated; the [256, 4000] similarity tile per core is computed
    as 2x8x16 PE matmuls (K=2048 accumulated in PSUM), exp applied on the
    scalar engine straight out of PSUM with a per-sample 1/(T*||f_i||) scale.
  - Because the bank is label-major with C=8 cams, every mask in the loss is a
    static stride pattern: intra-cam denominators are per-residue (mod 8)
    sums, the same-label sums are per-8-block sums, and the first-50
    hard-negative sum is a prefix over global columns [0,50)/[0,58) (core 0).
    All are strided vector-engine reductions - no gathers on device.
  - The own-logit numerator is a per-sample dot with its own center (host
    gathers the 256 own centers, 32 samples' worth per core).
  - The tiny [256]-sized tail (log, segment means over labels/cams) runs on
    the host at gather time.

Raw Bass (nc.Block) is used instead of the Tile framework: the installed
walrus rejects two raw-ISA instructions Tile's exit barrier emits
(EVENT_SEMAPHORE_RANGE_CLEAR, multi-wait DRAIN) and InstTensorTensorReduce.
"""

import numpy as np
import ml_dtypes
from contextlib import ExitStack

import concourse.bass as bass
from concourse import mybir
from concourse.bass_utils import run_bass_kernel_spmd

# problem constants (hardcoded per harness contract)
N, D, M = 256, 2048, 32000
L, C = 4000, 8
T = 0.07
LAMDA = 0.5
NCORES = 8
SHARD = M // NCORES          # 4000 centers per core
LBL_SHARD = SHARD // C       # 500 labels per core
CHUNK = 500                  # matmul moving free dim; 8 chunks per shard
NCHUNKS = SHARD // CHUNK     # 8
QUARTER = SHARD // 4         # 1000 cols = 125 whole label blocks
KT = D // 128                # 16 k-tiles
NS = N // NCORES             # 32 samples per core for the own-logit dot
NSLAB = 4                    # slab ring depth

F32 = mybir.dt.float32
BF16 = mybir.dt.bfloat16
ADD = mybir.AluOpType.add
AX = mybir.AxisListType.X
EXP = mybir.ActivationFunctionType.Exp


SQUARE = mybir.ActivationFunctionType.Square
F16 = mybir.dt.float16
NPSUM = 4                    # psum bank pairs: PE runs up to 4 chunks ahead of exp
NWARM = 24                   # dummy matmuls to warm the PE clock before chunk 0
W_FULL = 512                 # chunk width (64 whole labels, 0 mod 8)
W_LAST = SHARD - 7 * W_FULL  # 416 (52 whole labels)
CW = [W_FULL] * 7 + [W_LAST]
# layout of the consolidated small output [128, 2, 68] per m:
#   cols 8n+r (n<8, r<8) = per-chunk camera-residue exp sums (512 = 0 mod 8,
#       so chunk-local residue == global residue; host just sums chunks)
#   cols 64:66 = prefix sums P50, P58 (host uses core 0's)
#   col  66    = per-sample feat norm ||f_i||
#   col  67    = own-dot (raw <f_i, own_center_i>), rows 0:32 of m=0 only
SM_W = 68


def _build_program() -> bass.Bass:
    nc = bass.Bass()
    cTa = nc.dram_tensor("cTa", [7, 128, KT, W_FULL], BF16, kind="ExternalInput")
    cTb = nc.dram_tensor("cTb", [128, KT, W_LAST], BF16, kind="ExternalInput")
    fT = nc.dram_tensor("fT", [128, KT, N], BF16, kind="ExternalInput")
    fhd = nc.dram_tensor("feats16", [2, 128, D], F16, kind="ExternalInput")
    fsd = nc.dram_tensor("fs16", [NS, D], F16, kind="ExternalInput")
    ocd = nc.dram_tensor("oc16", [NS, D], F16, kind="ExternalInput")
    sm_out = nc.dram_tensor("SM_out", [128, 2, SM_W], F32, kind="ExternalOutput")
    bs_out = nc.dram_tensor("BS_out", [2, 128, LBL_SHARD], F32,
                            kind="ExternalOutput")

    with ExitStack() as ctx:
        e = ctx.enter_context

        ft_sb = e(nc.sbuf_tensor("ft_sb", [128, KT, N], BF16))
        slabs = [e(nc.sbuf_tensor(f"slab{j}", [128, KT, W_FULL], BF16))
                 for j in range(NSLAB)]
        et = [e(nc.sbuf_tensor(f"e{m}", [128, SHARD], F32)) for m in range(2)]
        fh_sb = e(nc.sbuf_tensor("fh_sb", [128, 2, D], F16))
        sq = e(nc.sbuf_tensor("sq", [128, D], F32))
        fs_sb = e(nc.sbuf_tensor("fs_sb", [NS, D], F16))
        oc_sb = e(nc.sbuf_tensor("oc_sb", [NS, D], F16))
        scr = e(nc.sbuf_tensor("scr", [NS, D], F32))

        ssum = [e(nc.sbuf_tensor(f"ssum{m}", [128, 1], F32)) for m in range(2)]
        inv = [e(nc.sbuf_tensor(f"inv{m}", [128, 1], F32)) for m in range(2)]
        sv = [e(nc.sbuf_tensor(f"sv{m}", [128, 1], F32)) for m in range(2)]

        bs = [e(nc.sbuf_tensor(f"bs{m}", [128, LBL_SHARD], F32)) for m in range(2)]
        small = e(nc.sbuf_tensor("small", [128, 2, SM_W], F32))

        ps = [[e(nc.psum_tensor(f"ps{b}_{m}", [128, W_FULL], F32))
               for m in range(2)] for b in range(NPSUM)]

        sem_ft = e(nc.semaphore("sem_ft"))
        sem_ftb = e(nc.semaphore("sem_ftb"))
        sem_slab = [e(nc.semaphore(f"sem_slab{j}")) for j in range(NSLAB)]
        sem_slab0b = e(nc.semaphore("sem_slab0b"))
        sem_f16 = e(nc.semaphore("sem_f16"))
        sem_fso = e(nc.semaphore("sem_fso"))
        sem_pe = e(nc.semaphore("sem_pe"))
        sem_act = e(nc.semaphore("sem_act"))
        c_a = e(nc.semaphore("c_a"))       # ACT prologue progress
        c_v = e(nc.semaphore("c_v"))       # DVE progress: every vector op incs
        c_warm = e(nc.semaphore("c_warm"))
        sem_od = e(nc.semaphore("sem_od"))

        # DVE instruction indices (c_v values after each op)
        V_SV = 5              # sv0 and sv1 both written
        V_P = 11              # dot + p50/58 done
        V_HALF = V_P + 4 * 4  # chunk 0..3 reductions done
        V_LAST = V_P + 8 * 4  # all chunk reductions done

        block = e(nc.Block(no_gpsimd_drain=True))

        @block.sync
        def _(sync):
            # first ft half, first slab0 half: minimal path to the first matmul
            sync.dma_start(out=ft_sb[:, 0:8, :], in_=fT[:, 0:8, :]).then_inc(
                sem_ft, 16)
            sync.dma_start(out=slabs[0][:, 0:8, :],
                           in_=cTa[0, :, 0:8, :]).then_inc(sem_slab[0], 16)
            sync.dma_start(out=ft_sb[:, 8:16, :], in_=fT[:, 8:16, :]).then_inc(
                sem_ftb, 16)
            sync.dma_start(out=slabs[0][:, 8:16, :],
                           in_=cTa[0, :, 8:16, :]).then_inc(sem_slab0b, 16)
            for n in range(1, NCHUNKS):
                j = n % NSLAB
                if n >= NSLAB:
                    # slot free once PE finished chunk n-NSLAB
                    sync.wait_ge(sem_pe, n - NSLAB + 1)
                if n < 7:
                    sync.dma_start(out=slabs[j][:, :, :], in_=cTa[n]).then_inc(
                        sem_slab[j], 16)
                else:
                    sync.dma_start(out=slabs[j][:, :, 0:W_LAST],
                                   in_=cTb[:, :, :]).then_inc(sem_slab[j], 16)
            # early writeback of the first four chunks' label-block sums
            sync.wait_ge(c_v, V_HALF)
            sync.dma_start(out=bs_out[0][:, 0:256], in_=bs[0][:, 0:256]).then_inc(
                sem_od, 16)
            sync.dma_start(out=bs_out[1][:, 0:256], in_=bs[1][:, 0:256]).then_inc(
                sem_od, 16)
            # final writeback
            sync.wait_ge(c_v, V_LAST)
            sync.dma_start(out=sm_out[:, :, :], in_=small[:, :, :]).then_inc(
                sem_od, 16)
            sync.dma_start(out=bs_out[0][:, 256:500],
                           in_=bs[0][:, 256:500]).then_inc(sem_od, 16)
            sync.dma_start(out=bs_out[1][:, 256:500],
                           in_=bs[1][:, 256:500]).then_inc(sem_od, 16)
            sync.wait_ge(sem_od, 80)

        @block.tensor
        def _(tensor):
            tensor.wait_ge(sem_ft, 16)
            # dummy matmuls on the already-loaded ft half: warms the PE clock
            # gate (HAM) while the first center slab is still in flight
            last = None
            for w in range(NWARM):
                last = tensor.matmul(ps[NPSUM - 1][0][:, 0:N],
                                     ft_sb[:, 0, 0:128], ft_sb[:, 0, :],
                                     start=True, stop=True)
            last.then_inc(c_warm, 1)
            slot_seen = [0] * NSLAB
            for n in range(NCHUNKS):
                j = n % NSLAB
                b = n % NPSUM
                w = CW[n]
                if n == 0:
                    tensor.wait_ge(sem_slab[0], 16)   # first half only
                    slot_seen[0] = 16
                else:
                    slot_seen[j] += 16
                    tensor.wait_ge(sem_slab[j], slot_seen[j])
                if n >= NPSUM:
                    # psum bank pair free once ACT consumed chunk n-NPSUM
                    tensor.wait_ge(sem_act, 2 * (n - NPSUM + 1))
                if n == NPSUM - 1:
                    # warmup dummies wrote this psum bank (WAW ordering)
                    tensor.wait_ge(c_warm, 1)
                last = None
                for ki in range(KT):
                    if n == 0 and ki == 8:
                        tensor.wait_ge(sem_ftb, 16)
                        tensor.wait_ge(sem_slab0b, 16)
                    for m in range(2):
                        last = tensor.matmul(
                            ps[b][m][:, 0:w],
                            ft_sb[:, ki, m * 128:(m + 1) * 128],
                            slabs[j][:, ki, 0:w],
                            start=(ki == 0), stop=(ki == KT - 1))
                last.then_inc(sem_pe, 1)

        @block.scalar
        def _(scalar):
            # setup inputs ride the ACT engine's own HW-DGE ring, in parallel
            # with the sync ring's ft/slab stream
            scalar.dma_start(
                out=fh_sb[:, :, :],
                in_=fhd.rearrange("m p d -> p m d")).then_inc(sem_f16, 16)
            scalar.dma_start(out=fs_sb[:, :], in_=fsd[:, :]).then_inc(sem_fso, 16)
            scalar.dma_start(out=oc_sb[:, :], in_=ocd[:, :]).then_inc(sem_fso, 16)
            # row sums-of-squares + norms for the exp scale (ACT-only prologue)
            scalar.wait_ge(sem_f16, 16)
            for m in range(2):
                scalar.activation(out=sq[:, :], in_=fh_sb[:, m, :], func=SQUARE,
                                  accum_out=ssum[m][:, :]).then_inc(c_a, 1)
                scalar.wait_ge(c_a, 2 * m + 1)
                scalar.sqrt(small[:, m, 66:67], ssum[m][:, :]).then_inc(c_a, 1)
            # exp stream straight out of PSUM with per-sample scale
            scalar.wait_ge(c_v, V_SV)
            for n in range(NCHUNKS):
                b = n % NPSUM
                w = CW[n]
                scalar.wait_ge(sem_pe, n + 1)
                for m in range(2):
                    scalar.activation(
                        out=et[m][:, n * W_FULL:n * W_FULL + w],
                        in_=ps[b][m][:, 0:w],
                        func=EXP, scale=sv[m][:, :]).then_inc(sem_act, 1)

        @block.vector
        def _(vector):
            vcount = 0

            def v(instr):
                nonlocal vcount
                instr.then_inc(c_v, 1)
                vcount += 1
                return vcount

            # zero the never-fully-written column of `small` (DMA'd out whole);
            # the dot-reduce overwrites rows 0:32 of m=0 later, in order
            v(vector.memset(small[:, :, 67:68], 0.0))              # op 1
            for m in range(2):                                     # ops 2..5
                vector.wait_ge(c_a, 2 * (m + 1))
                v(vector.reciprocal(inv[m][:, :], small[:, m, 66:67]))
                vector.wait_ge(c_v, vcount)
                v(vector.tensor_scalar_mul(sv[m][:, :], inv[m][:, :], 1.0 / T))
            assert vcount == V_SV
            # raw own-logit dot (host divides by T*norm at gather time)
            vector.wait_ge(sem_fso, 32)
            v(vector.tensor_mul(scr[:, :], fs_sb[:, :], oc_sb[:, :]))   # 6
            vector.wait_ge(c_v, vcount)
            v(vector.tensor_reduce(out=small[0:NS, 0, 67:68], in_=scr[:, :],  # 7
                                   axis=AX, op=ADD))
            # prefix sums over global columns [0,50)/[0,58) (host uses core 0's)
            vector.wait_ge(sem_act, 2)
            for m in range(2):                                     # ops 8..11
                v(vector.tensor_reduce(out=small[:, m, 64:65], in_=et[m][:, 0:50],
                                       axis=AX, op=ADD))
                v(vector.tensor_reduce(out=small[:, m, 65:66], in_=et[m][:, 0:58],
                                       axis=AX, op=ADD))
            assert vcount == V_P
            # per-chunk reductions right behind each exp: label-block sums and
            # camera-residue sums (chunks are 0 mod 8 wide -> fully aligned)
            for n in range(NCHUNKS):                               # 4 ops/chunk
                w = CW[n]
                nl = w // C                                        # 64 or 52
                vector.wait_ge(sem_act, 2 * (n + 1))
                for m in range(2):
                    chunk = et[m][:, n * W_FULL:n * W_FULL + w]
                    v(vector.tensor_reduce(
                        out=bs[m][:, 64 * n:64 * n + nl],
                        in_=chunk.rearrange("p (l r) -> p l r", r=C),
                        axis=AX, op=ADD))
                    v(vector.tensor_reduce(
                        out=small[:, m, 8 * n:8 * n + 8],
                        in_=chunk.rearrange("p (l r) -> p r l", r=C),
                        axis=AX, op=ADD))
            assert vcount == V_LAST

    return nc


_PROGRAM_CACHE: dict[str, bass.Bass] = {}


def _program() -> bass.Bass:
    if "nc" not in _PROGRAM_CACHE:
        _PROGRAM_CACHE["nc"] = _build_program()
    return _PROGRAM_CACHE["nc"]


def _make_in_maps(feats, centers, own_centers):
    bf = ml_dtypes.bfloat16
    fT_host = np.ascontiguousarray(feats.T)            # [2048, 256] f32
    fT_bf = fT_host.astype(bf).reshape(KT, 128, N).transpose(1, 0, 2)
    fT_bf = np.ascontiguousarray(fT_bf)                # [128, 16, 256]
    fh_host = feats.astype(np.float16).reshape(2, 128, D)
    cT_all = np.ascontiguousarray(centers.T).astype(bf)  # [2048, 32000] bf16

    in_maps = []
    for c in range(NCORES):
        shard = cT_all[:, c * SHARD:(c + 1) * SHARD]     # [2048, 4000]
        sk = shard.reshape(KT, 128, SHARD)               # [16, 128, 4000]
        a = sk[:, :, 0:7 * W_FULL].reshape(KT, 128, 7, W_FULL)
        a = np.ascontiguousarray(a.transpose(2, 1, 0, 3))  # [7, 128, 16, 512]
        b = np.ascontiguousarray(
            sk[:, :, 7 * W_FULL:].transpose(1, 0, 2))      # [128, 16, 416]
        in_maps.append({
            "cTa": a,
            "cTb": b,
            "fT": fT_bf,
            "feats16": fh_host,
            "fs16": np.ascontiguousarray(
                feats[c * NS:(c + 1) * NS].astype(np.float16)),
            "oc16": np.ascontiguousarray(
                own_centers[c * NS:(c + 1) * NS].astype(np.float16)),
        })
    return in_maps


def _host_tail(results, labels, camids, epoch):
    n = labels.shape[0]
    # SM_out [128, 2, SM_W]: sample i lives at [i % 128, i // 128, :]
    SM = [r["SM_out"].transpose(1, 0, 2).reshape(n, SM_W) for r in results]
    # per-chunk camera-residue sums (aligned: just sum over chunks and cores)
    S = np.zeros((n, C), np.float32)
    for sm in SM:
        S += sm[:, 0:64].reshape(n, NCHUNKS, C).sum(axis=1)
    denom_intra = S[np.arange(n), camids]

    owner = (labels // LBL_SHARD).astype(np.int64)
    BS = np.stack([r["BS_out"].reshape(n, LBL_SHARD) for r in results])
    B = BS[owner, np.arange(n), labels % LBL_SHARD]
    p50, p58 = SM[0][:, 64], SM[0][:, 65]
    hard = np.where(labels <= 6, p58 - B, p50)
    denom_inter = B + hard

    nrm = SM[0][:, 66]                                # replicated across cores
    dot = np.concatenate([r["SM_out"][0:NS, 0, 67] for r in results])  # [n]
    own = dot / (T * nrm)

    loss_i = own - np.log(denom_intra)
    loss_j = own - np.log(denom_inter)

    cam_sums = np.zeros(C, np.float32)
    cam_cnts = np.zeros(C, np.float32)
    np.add.at(cam_sums, camids, loss_i)
    np.add.at(cam_cnts, camids, 1.0)
    loss_intra = -np.sum(
        np.where(cam_cnts > 0, cam_sums / np.maximum(cam_cnts, 1.0), 0.0),
        dtype=np.float32)

    lbl_sums = np.zeros(L, np.float32)
    lbl_cnts = np.zeros(L, np.float32)
    np.add.at(lbl_sums, labels, loss_j)
    np.add.at(lbl_cnts, labels, 1.0)
    loss_inter = -np.sum(
        np.where(lbl_cnts > 0, lbl_sums / np.maximum(lbl_cnts, 1.0), 0.0),
        dtype=np.float32)

    if int(epoch) < 5:
        return np.float32(loss_intra)
    return np.stack([loss_intra, LAMDA * loss_inter]).astype(np.float32)


def kernel(feats, centers, labels, camids, epoch):
    feats = np.ascontiguousarray(np.asarray(feats, dtype=np.float32))
    centers = np.ascontiguousarray(np.asarray(centers, dtype=np.float32))
    labels = np.asarray(labels).astype(np.int64)
    camids = np.asarray(camids).astype(np.int64)

    own_idx = labels * C + camids
    own_centers = centers[own_idx]                     # host gather [256, 2048]

    in_maps = _make_in_maps(feats, centers, own_centers)
    res = run_bass_kernel_spmd(_program(), in_maps, list(range(NCORES))).results
    return _host_tail(res, labels, camids, epoch)



# revision 4
# speedup vs baseline: 1.5386x; 1.5386x over previous
"""Trainium2 Bass kernel for the CAP loss (camera-aware proxy memory bank).

Strategy (8 NeuronCores, SPMD, raw Bass engine blocks), v2 = fp8 DoubleRow:
  - The center bank [32000, 2048] is sharded along the center axis: 4000
    centers (= 500 labels x 8 cams, label-major) per core, pre-transposed,
    scaled by SC and cast to fp8(e4m3) on the host so each core streams a
    [2048, 4000] fp8 shard as 8 fully-contiguous ~1MB slabs.
  - feats are replicated, scaled by SF, fp8.  The [256, 4000] similarity tile
    per core is computed with DoubleRow fp8 matmuls: each PE instruction
    contracts TWO 128-deep k-tiles (stationary [128,2,128] fp8, moving
    [128,2,512] fp8) at double rate, K=2048 accumulated in PSUM over 8
    instruction pairs.  exp is applied on the scalar engine straight out of
    PSUM with a host-computed per-sample scale 1/(T*||f_i||*SF*SC), output in
    bf16 (halves DVE read traffic).
  - Because the bank is label-major with C=8 cams, every mask in the loss is a
    static stride pattern: intra-cam denominators are per-residue (mod 8)
    sums, the same-label sums are per-8-block sums, and the first-50
    hard-negative sum is a prefix over global columns [0,50)/[0,58) (core 0).
    All are strided vector-engine reductions - no gathers on device.
  - The own-logit numerator and the tiny [256]-sized tail (log, segment means
    over labels/cams) run on the host (microscopic: 256 dots + segment means).
  - Label-block sums are written back incrementally per chunk from the DVE's
    own DMA ring, so the post-matmul tail is just the last chunk's epilogue.

Raw Bass (nc.Block) is used instead of the Tile framework: the installed
walrus rejects two raw-ISA instructions Tile's exit barrier emits
(EVENT_SEMAPHORE_RANGE_CLEAR, multi-wait DRAIN) and InstTensorTensorReduce.
"""

import numpy as np
import ml_dtypes
from contextlib import ExitStack

import concourse.bass as bass
from concourse import mybir
from concourse.bass_utils import run_bass_kernel_spmd

# problem constants (hardcoded per harness contract)
N, D, M = 256, 2048, 32000
L, C = 4000, 8
T = 0.07
LAMDA = 0.5
NCORES = 8
SHARD = M // NCORES          # 4000 centers per core
LBL_SHARD = SHARD // C       # 500 labels per core
KT = D // 128                # 16 k-tiles
KPAIR = KT // 2              # 8 DoubleRow k-tile pairs
NSLAB = 4                    # slab ring depth
NPSUM = 4                    # psum bank pairs: PE runs up to 4 chunks ahead
NWARM = 48                   # dummy matmuls to warm the PE clock before chunk 0
W_FULL = 512                 # chunk width (64 whole labels, 0 mod 8)
W_LAST = SHARD - 7 * W_FULL  # 416 (52 whole labels)
NCHUNKS = 8
CW = [W_FULL] * 7 + [W_LAST]
SF = 32.0                    # feats fp8 pre-scale
SC = 1024.0                  # centers fp8 pre-scale
# layout of the consolidated small output [128, 2, 66] per m:
#   cols 8n+r (n<8, r<8) = per-chunk camera-residue exp sums (512 = 0 mod 8,
#       so chunk-local residue == global residue; host just sums chunks)
#   cols 64:66 = prefix sums P50, P58 (host uses core 0's)
SM_W = 66

F32 = mybir.dt.float32
BF16 = mybir.dt.bfloat16
FP8 = mybir.dt.float8e4
ADD = mybir.AluOpType.add
AX = mybir.AxisListType.X
EXP = mybir.ActivationFunctionType.Exp
DROW = mybir.MatmulPerfMode.DoubleRow


def _build_program() -> bass.Bass:
    nc = bass.Bass()
    cTa = nc.dram_tensor("cTa", [7, 128, KT, W_FULL], FP8, kind="ExternalInput")
    cTb = nc.dram_tensor("cTb", [128, KT, W_LAST], FP8, kind="ExternalInput")
    fT = nc.dram_tensor("fT", [128, KT, N], FP8, kind="ExternalInput")
    svd = nc.dram_tensor("svd", [128, 2], F32, kind="ExternalInput")
    sm_out = nc.dram_tensor("SM_out", [128, 2, SM_W], F32, kind="ExternalOutput")
    bs_out = nc.dram_tensor("BS_out", [2, 128, LBL_SHARD], F32,
                            kind="ExternalOutput")

    with ExitStack() as ctx:
        e = ctx.enter_context

        ft_sb = e(nc.sbuf_tensor("ft_sb", [128, KT, N], FP8))
        slabs = [e(nc.sbuf_tensor(f"slab{j}", [128, KT, W_FULL], FP8))
                 for j in range(NSLAB)]
        et = [e(nc.sbuf_tensor(f"e{m}", [128, SHARD], BF16)) for m in range(2)]
        sv_sb = e(nc.sbuf_tensor("sv_sb", [128, 2], F32))
        bs = [e(nc.sbuf_tensor(f"bs{m}", [128, LBL_SHARD], F32)) for m in range(2)]
        small = e(nc.sbuf_tensor("small", [128, 2, SM_W], F32))

        ps = [[e(nc.psum_tensor(f"ps{b}_{m}", [128, W_FULL], F32))
               for m in range(2)] for b in range(NPSUM)]

        sem_ft = e(nc.semaphore("sem_ft"))
        sem_ftb = e(nc.semaphore("sem_ftb"))
        sem_slab = [e(nc.semaphore(f"sem_slab{j}")) for j in range(NSLAB)]
        sem_slab0b = e(nc.semaphore("sem_slab0b"))
        sem_sv = e(nc.semaphore("sem_sv"))
        sem_pe = e(nc.semaphore("sem_pe"))
        sem_act = e(nc.semaphore("sem_act"))
        c_v = e(nc.semaphore("c_v"))       # DVE progress: every vector op incs
        c_warm = e(nc.semaphore("c_warm"))
        sem_od = e(nc.semaphore("sem_od"))

        N_WB = 2 * NCHUNKS + 1             # bs per-chunk x2 + final small

        block = e(nc.Block(no_gpsimd_drain=True))

        @block.sync
        def _(sync):
            # first ft half, first slab0 half: minimal path to the first matmul
            sync.dma_start(out=ft_sb[:, 0:8, :], in_=fT[:, 0:8, :]).then_inc(
                sem_ft, 16)
            sync.dma_start(out=slabs[0][:, 0:8, :],
                           in_=cTa[0, :, 0:8, :]).then_inc(sem_slab[0], 16)
            sync.dma_start(out=ft_sb[:, 8:16, :], in_=fT[:, 8:16, :]).then_inc(
                sem_ftb, 16)
            sync.dma_start(out=slabs[0][:, 8:16, :],
                           in_=cTa[0, :, 8:16, :]).then_inc(sem_slab0b, 16)
            wb = 0                         # next chunk whose bs cols to write

            def write_back(sync, n):
                # chunk n's bs columns are final once DVE op 4+4(n+1) retired
                sync.wait_ge(c_v, 4 + 4 * (n + 1))
                nl = CW[n] // C
                for m in range(2):
                    sync.dma_start(
                        out=bs_out[m][:, 64 * n:64 * n + nl],
                        in_=bs[m][:, 64 * n:64 * n + nl]).then_inc(sem_od, 16)

            for n in range(1, NCHUNKS):
                j = n % NSLAB
                if n >= NSLAB:
                    # slot free once PE finished chunk n-NSLAB
                    sync.wait_ge(sem_pe, n - NSLAB + 1)
                if n < 7:
                    sync.dma_start(out=slabs[j][:, :, :], in_=cTa[n]).then_inc(
                        sem_slab[j], 16)
                else:
                    sync.dma_start(out=slabs[j][:, :, 0:W_LAST],
                                   in_=cTb[:, :, :]).then_inc(sem_slab[j], 16)
                if n >= NSLAB + 1:
                    # interleave finished-chunk writebacks between slab issues:
                    # chunk n-5's DVE epilogue is long done by the time the
                    # slab-n issue gate (PE chunk n-4) clears
                    write_back(sync, wb)
                    wb += 1
            while wb < NCHUNKS:
                write_back(sync, wb)
                wb += 1
            sync.wait_ge(c_v, 4 + 4 * NCHUNKS)
            sync.dma_start(out=sm_out[:, :, :], in_=small[:, :, :]).then_inc(
                sem_od, 16)
            sync.wait_ge(sem_od, 16 * N_WB)

        @block.tensor
        def _(tensor):
            tensor.wait_ge(sem_ft, 16)
            # dummy matmuls on the already-loaded ft half: warms the PE clock
            # gate (HAM) while the first center slab is still in flight
            last = None
            for w in range(NWARM):
                last = tensor.matmul(ps[NPSUM - 1][0][:, 0:N],
                                     ft_sb[:, 0:2, 0:128], ft_sb[:, 0:2, :],
                                     start=True, stop=True, perf_mode=DROW)
            last.then_inc(c_warm, 1)
            slot_seen = [0] * NSLAB
            for n in range(NCHUNKS):
                j = n % NSLAB
                b = n % NPSUM
                w = CW[n]
                if n == 0:
                    tensor.wait_ge(sem_slab[0], 16)   # first half only
                    slot_seen[0] = 16
                else:
                    slot_seen[j] += 16
                    tensor.wait_ge(sem_slab[j], slot_seen[j])
                if n >= NPSUM:
                    # psum bank pair free once ACT consumed chunk n-NPSUM
                    tensor.wait_ge(sem_act, 2 * (n - NPSUM + 1))
                if n == NPSUM - 1:
                    # warmup dummies wrote this psum bank (WAW ordering)
                    tensor.wait_ge(c_warm, 1)
                last = None
                for kp in range(KPAIR):
                    if n == 0 and kp == 4:
                        tensor.wait_ge(sem_ftb, 16)
                        tensor.wait_ge(sem_slab0b, 16)
                    for m in range(2):
                        last = tensor.matmul(
                            ps[b][m][:, 0:w],
                            ft_sb[:, 2 * kp:2 * kp + 2, m * 128:(m + 1) * 128],
                            slabs[j][:, 2 * kp:2 * kp + 2, 0:w],
                            start=(kp == 0), stop=(kp == KPAIR - 1),
                            perf_mode=DROW)
                last.then_inc(sem_pe, 1)

        @block.scalar
        def _(scalar):
            # the tiny sv input rides the ACT engine's own HW-DGE ring
            scalar.dma_start(out=sv_sb[:, :], in_=svd[:, :]).then_inc(sem_sv, 16)
            scalar.wait_ge(sem_sv, 16)
            # exp stream straight out of PSUM with per-sample scale, bf16 out
            for n in range(NCHUNKS):
                b = n % NPSUM
                w = CW[n]
                scalar.wait_ge(sem_pe, n + 1)
                for m in range(2):
                    scalar.activation(
                        out=et[m][:, n * W_FULL:n * W_FULL + w],
                        in_=ps[b][m][:, 0:w],
                        func=EXP, scale=sv_sb[:, m:m + 1]).then_inc(sem_act, 1)

        @block.vector
        def _(vector):
            vcount = 0

            def v(instr):
                nonlocal vcount
                instr.then_inc(c_v, 1)
                vcount += 1
                return vcount

            # prefix sums over global columns [0,50)/[0,58) (host uses core 0's)
            vector.wait_ge(sem_act, 2)
            for m in range(2):
                v(vector.tensor_reduce(out=small[:, m, 64:65], in_=et[m][:, 0:50],
                                       axis=AX, op=ADD))
                v(vector.tensor_reduce(out=small[:, m, 65:66], in_=et[m][:, 0:58],
                                       axis=AX, op=ADD))
            # per-chunk reductions right behind each exp: label-block sums and
            # camera-residue sums (chunks are 0 mod 8 wide -> fully aligned),
            # then immediate writeback of the finished bs columns
            for n in range(NCHUNKS):
                w = CW[n]
                nl = w // C                                        # 64 or 52
                vector.wait_ge(sem_act, 2 * (n + 1))
                for m in range(2):
                    chunk = et[m][:, n * W_FULL:n * W_FULL + w]
                    v(vector.tensor_reduce(
                        out=bs[m][:, 64 * n:64 * n + nl],
                        in_=chunk.rearrange("p (l r) -> p l r", r=C),
                        axis=AX, op=ADD))
                    v(vector.tensor_reduce(
                        out=small[:, m, 8 * n:8 * n + 8],
                        in_=chunk.rearrange("p (l r) -> p r l", r=C),
                        axis=AX, op=ADD))


    return nc


_PROGRAM_CACHE: dict[str, bass.Bass] = {}


def _program() -> bass.Bass:
    if "nc" not in _PROGRAM_CACHE:
        _PROGRAM_CACHE["nc"] = _build_program()
    return _PROGRAM_CACHE["nc"]


def _make_in_maps(feats, centers, norms):
    f8 = ml_dtypes.float8_e4m3
    fT_host = np.ascontiguousarray(feats.T)            # [2048, 256] f32
    fT8 = np.clip(fT_host * SF, -240.0, 240.0).astype(f8)
    fT8 = np.ascontiguousarray(fT8.reshape(KT, 128, N).transpose(1, 0, 2))
    sv = (1.0 / (T * norms * SF * SC)).astype(np.float32)
    sv = np.ascontiguousarray(sv.reshape(2, 128).T)    # [128, 2], sample m*128+p
    cT8 = np.clip(np.ascontiguousarray(centers.T) * SC,
                  -240.0, 240.0).astype(f8)            # [2048, 32000] fp8

    in_maps = []
    for c in range(NCORES):
        shard = cT8[:, c * SHARD:(c + 1) * SHARD]        # [2048, 4000]
        sk = shard.reshape(KT, 128, SHARD)               # [16, 128, 4000]
        a = sk[:, :, 0:7 * W_FULL].reshape(KT, 128, 7, W_FULL)
        a = np.ascontiguousarray(a.transpose(2, 1, 0, 3))  # [7, 128, 16, 512]
        b = np.ascontiguousarray(
            sk[:, :, 7 * W_FULL:].transpose(1, 0, 2))      # [128, 16, 416]
        in_maps.append({"cTa": a, "cTb": b, "fT": fT8, "svd": sv})
    return in_maps


def _host_tail(results, labels, camids, epoch, own):
    n = labels.shape[0]
    # SM_out [128, 2, SM_W]: sample i lives at [i % 128, i // 128, :]
    SM = [r["SM_out"].transpose(1, 0, 2).reshape(n, SM_W) for r in results]
    # per-chunk camera-residue sums (aligned: just sum over chunks and cores)
    S = np.zeros((n, C), np.float32)
    for sm in SM:
        S += sm[:, 0:64].reshape(n, NCHUNKS, C).sum(axis=1)
    denom_intra = S[np.arange(n), camids]

    owner = (labels // LBL_SHARD).astype(np.int64)
    BS = np.stack([r["BS_out"].reshape(n, LBL_SHARD) for r in results])
    B = BS[owner, np.arange(n), labels % LBL_SHARD]
    p50, p58 = SM[0][:, 64], SM[0][:, 65]
    hard = np.where(labels <= 6, p58 - B, p50)
    denom_inter = B + hard

    loss_i = own - np.log(denom_intra)
    loss_j = own - np.log(denom_inter)

    cam_sums = np.zeros(C, np.float32)
    cam_cnts = np.zeros(C, np.float32)
    np.add.at(cam_sums, camids, loss_i.astype(np.float32))
    np.add.at(cam_cnts, camids, 1.0)
    loss_intra = -np.sum(
        np.where(cam_cnts > 0, cam_sums / np.maximum(cam_cnts, 1.0), 0.0),
        dtype=np.float32)

    lbl_sums = np.zeros(L, np.float32)
    lbl_cnts = np.zeros(L, np.float32)
    np.add.at(lbl_sums, labels, loss_j.astype(np.float32))
    np.add.at(lbl_cnts, labels, 1.0)
    loss_inter = -np.sum(
        np.where(lbl_cnts > 0, lbl_sums / np.maximum(lbl_cnts, 1.0), 0.0),
        dtype=np.float32)

    if int(epoch) < 5:
        return np.float32(loss_intra)
    return np.stack([loss_intra, LAMDA * loss_inter]).astype(np.float32)


def kernel(feats, centers, labels, camids, epoch):
    feats = np.ascontiguousarray(np.asarray(feats, dtype=np.float32))
    centers = np.ascontiguousarray(np.asarray(centers, dtype=np.float32))
    labels = np.asarray(labels).astype(np.int64)
    camids = np.asarray(camids).astype(np.int64)

    norms = np.linalg.norm(feats.astype(np.float64), axis=1)
    own_idx = labels * C + camids
    own = np.einsum("ij,ij->i", feats.astype(np.float64),
                    centers[own_idx].astype(np.float64)) / (T * norms)

    in_maps = _make_in_maps(feats, centers, norms)
    res = run_bass_kernel_spmd(_program(), in_maps, list(range(NCORES))).results
    return _host_tail(res, labels, camids, epoch, own)


# revision 5
# speedup vs baseline: 1.6384x; 1.0648x over previous
"""Trainium2 Bass kernel for the CAP loss (camera-aware proxy memory bank).

Strategy (8 NeuronCores, SPMD, raw Bass engine blocks), v3 = fp8 DoubleRow:
  - The center bank [32000, 2048] is sharded along the center axis: 4000
    centers (= 500 labels x 8 cams, label-major) per core, pre-transposed,
    scaled by SC and cast to fp8(e4m3) on the host so each core streams a
    [2048, 4000] fp8 shard as 9 contiguous slabs (7x512 + 256 + 160 cols;
    the small final chunks shrink the serial post-matmul tail).
  - feats are replicated, row-normalized on the host, scaled by SF, fp8.
    The [256, 4000] similarity tile per core is computed with DoubleRow fp8
    matmuls: each PE instruction contracts TWO 128-deep k-tiles (stationary
    [128,2,128] fp8, moving [128,2,w] fp8) at double rate, K=2048 accumulated
    in PSUM over 8 instruction pairs.  Because feats are pre-normalized the
    exp scale is the compile-time constant 1/(T*SF*SC): exp is applied on the
    scalar engine straight out of a 2-bank PSUM pair (both 128-sample halves
    in one op), output in bf16 (halves DVE read traffic).
  - Because the bank is label-major with C=8 cams, every mask in the loss is a
    static stride pattern: intra-cam denominators are per-residue (mod 8)
    sums, the same-label sums are per-8-block sums, and the first-50
    hard-negative sum is a prefix over global columns [0,50)/[0,58) (core 0).
    All are strided vector-engine reductions - no gathers on device.
  - The own-logit numerator and the tiny [256]-sized tail (log, segment means
    over labels/cams) run on the host (microscopic: 256 dots + segment means).
  - Label-block sums are written back incrementally per chunk (sync ring,
    gated on DVE progress), so the tail is just the last 160-col epilogue.

Raw Bass (nc.Block) is used instead of the Tile framework: the installed
walrus rejects two raw-ISA instructions Tile's exit barrier emits
(EVENT_SEMAPHORE_RANGE_CLEAR, multi-wait DRAIN) and InstTensorTensorReduce.
"""

import numpy as np
import ml_dtypes
from contextlib import ExitStack

import concourse.bass as bass
from concourse import mybir
from concourse.bass_utils import run_bass_kernel_spmd

# problem constants (hardcoded per harness contract)
N, D, M = 256, 2048, 32000
L, C = 4000, 8
T = 0.07
LAMDA = 0.5
NCORES = 8
SHARD = M // NCORES          # 4000 centers per core
LBL_SHARD = SHARD // C       # 500 labels per core
KT = D // 128                # 16 k-tiles
KPAIR = KT // 2              # 8 DoubleRow k-tile pairs
NSLAB = 4                    # slab ring depth
NPSUM = 4                    # psum 2-bank pairs: PE runs up to 4 chunks ahead
NWARM = 10                   # dummy matmuls to warm the PE clock before chunk 0
W_FULL = 512
CW = [512] * 7 + [256, 160]  # chunk widths (all 0 mod 8)
CO = [0, 512, 1024, 1536, 2048, 2560, 3072, 3584, 3840]  # chunk col offsets
NCHUNKS = 9
SF = 1024.0                  # normalized-feats fp8 pre-scale
SC = 1024.0                  # centers fp8 pre-scale
ESCALE = 1.0 / (T * SF * SC)  # constant exp scale
# layout of the consolidated small output [128, 2, 74] per m:
#   cols 8n+r (n<9, r<8) = per-chunk camera-residue exp sums (chunks are
#       0 mod 8 wide, so chunk-local residue == global residue)
#   cols 72:74 = prefix sums P50, P58 (host uses core 0's)
SM_W = 74

F32 = mybir.dt.float32
BF16 = mybir.dt.bfloat16
FP8 = mybir.dt.float8e4
ADD = mybir.AluOpType.add
AX = mybir.AxisListType.X
EXP = mybir.ActivationFunctionType.Exp
DROW = mybir.MatmulPerfMode.DoubleRow


def _build_program() -> bass.Bass:
    nc = bass.Bass()
    cTa = nc.dram_tensor("cTa", [7, 128, KT, W_FULL], FP8, kind="ExternalInput")
    cTb7 = nc.dram_tensor("cTb7", [128, KT, CW[7]], FP8, kind="ExternalInput")
    cTb8 = nc.dram_tensor("cTb8", [128, KT, CW[8]], FP8, kind="ExternalInput")
    fT = nc.dram_tensor("fT", [128, KT, N], FP8, kind="ExternalInput")
    sm_out = nc.dram_tensor("SM_out", [128, 2, SM_W], F32, kind="ExternalOutput")
    bs_out = nc.dram_tensor("BS_out", [128, 2, LBL_SHARD], F32,
                            kind="ExternalOutput")

    with ExitStack() as ctx:
        e = ctx.enter_context

        ft_sb = e(nc.sbuf_tensor("ft_sb", [128, KT, N], FP8))
        slabs = [e(nc.sbuf_tensor(f"slab{j}", [128, KT, W_FULL], FP8))
                 for j in range(NSLAB)]
        et = e(nc.sbuf_tensor("et", [128, 2, SHARD], BF16))
        bs = e(nc.sbuf_tensor("bs", [128, 2, LBL_SHARD], F32))
        small = e(nc.sbuf_tensor("small", [128, 2, SM_W], F32))
        scr = e(nc.sbuf_tensor("scr", [128, 2], F32))

        # each ps[b] is a 2-bank pair: cols 0:512 = samples 0:128 (m=0),
        # cols 512:1024 = samples 128:256 (m=1); exp consumes both in one op
        ps = [e(nc.psum_tensor(f"ps{b}", [128, 2 * W_FULL], F32))
              for b in range(NPSUM)]

        sem_ft = e(nc.semaphore("sem_ft"))       # fT k-tiles 0:2
        sem_ftb = e(nc.semaphore("sem_ftb"))     # fT k-tiles 2:8
        sem_ftc = e(nc.semaphore("sem_ftc"))     # fT k-tiles 8:16
        sem_slab = [e(nc.semaphore(f"sem_slab{j}")) for j in range(NSLAB)]
        sem_slab0b = e(nc.semaphore("sem_slab0b"))
        sem_pe = e(nc.semaphore("sem_pe"))
        sem_act = e(nc.semaphore("sem_act"))
        c_v = e(nc.semaphore("c_v"))       # DVE progress: every vector op incs
        c_warm = e(nc.semaphore("c_warm"))
        sem_od = e(nc.semaphore("sem_od"))

        N_WB = NCHUNKS + 1                 # bs per-chunk + final small

        block = e(nc.Block(no_gpsimd_drain=True))

        @block.sync
        def _(sync):
            # minimal path to the first matmul: 2 k-tiles of ft, then the
            # first slab half, with the rest of ft threaded between
            sync.dma_start(out=ft_sb[:, 0:2, :], in_=fT[:, 0:2, :]).then_inc(
                sem_ft, 16)
            sync.dma_start(out=slabs[0][:, 0:8, :],
                           in_=cTa[0, :, 0:8, :]).then_inc(sem_slab[0], 16)
            sync.dma_start(out=ft_sb[:, 2:8, :], in_=fT[:, 2:8, :]).then_inc(
                sem_ftb, 16)
            sync.dma_start(out=slabs[0][:, 8:16, :],
                           in_=cTa[0, :, 8:16, :]).then_inc(sem_slab0b, 16)
            sync.dma_start(out=ft_sb[:, 8:16, :], in_=fT[:, 8:16, :]).then_inc(
                sem_ftc, 16)

            wb = 0                         # next chunk whose bs cols to write

            def write_back(n):
                # chunk n's bs columns are final once DVE op 2+2(n+1) retired
                sync.wait_ge(c_v, 2 + 2 * (n + 1))
                nl = CW[n] // C
                co = CO[n] // C
                sync.dma_start(
                    out=bs_out[:, :, co:co + nl],
                    in_=bs[:, :, co:co + nl]).then_inc(sem_od, 16)

            for n in range(1, NCHUNKS):
                j = n % NSLAB
                if n >= NSLAB:
                    # slot free once PE finished chunk n-NSLAB
                    sync.wait_ge(sem_pe, n - NSLAB + 1)
                if n < 7:
                    sync.dma_start(out=slabs[j][:, :, :], in_=cTa[n]).then_inc(
                        sem_slab[j], 16)
                elif n == 7:
                    sync.dma_start(out=slabs[j][:, :, 0:CW[7]],
                                   in_=cTb7[:, :, :]).then_inc(sem_slab[j], 16)
                else:
                    sync.dma_start(out=slabs[j][:, :, 0:CW[8]],
                                   in_=cTb8[:, :, :]).then_inc(sem_slab[j], 16)
                if n >= NSLAB + 1:
                    # interleave finished-chunk writebacks between slab issues:
                    # chunk n-5's DVE epilogue is long done by the time the
                    # slab-n issue gate (PE chunk n-4) clears
                    write_back(wb)
                    wb += 1
            while wb < NCHUNKS:
                write_back(wb)
                wb += 1
            sync.wait_ge(c_v, 2 + 2 * NCHUNKS)
            sync.dma_start(out=sm_out[:, :, :], in_=small[:, :, :]).then_inc(
                sem_od, 16)
            sync.wait_ge(sem_od, 16 * N_WB)

        @block.tensor
        def _(tensor):
            tensor.wait_ge(sem_ft, 16)
            # dummy matmuls on the already-loaded ft k-tiles: warms the PE
            # clock gate (HAM) while the first center slab is still in flight
            last = None
            for w in range(NWARM):
                last = tensor.matmul(ps[NPSUM - 1][:, 0:N],
                                     ft_sb[:, 0:2, 0:128], ft_sb[:, 0:2, :],
                                     start=True, stop=True, perf_mode=DROW)
            last.then_inc(c_warm, 1)
            slot_seen = [0] * NSLAB
            for n in range(NCHUNKS):
                j = n % NSLAB
                b = n % NPSUM
                w = CW[n]
                if n == 0:
                    tensor.wait_ge(sem_slab[0], 16)   # first half only
                    slot_seen[0] = 16
                else:
                    slot_seen[j] += 16
                    tensor.wait_ge(sem_slab[j], slot_seen[j])
                if n >= NPSUM:
                    # psum bank pair free once ACT consumed chunk n-NPSUM
                    tensor.wait_ge(sem_act, n - NPSUM + 1)
                if n == NPSUM - 1:
                    # warmup dummies wrote this psum bank (WAW ordering)
                    tensor.wait_ge(c_warm, 1)
                last = None
                for kp in range(KPAIR):
                    if n == 0 and kp == 1:
                        tensor.wait_ge(sem_ftb, 16)
                    if n == 0 and kp == 4:
                        tensor.wait_ge(sem_ftc, 16)
                        tensor.wait_ge(sem_slab0b, 16)
                    for m in range(2):
                        last = tensor.matmul(
                            ps[b][:, m * W_FULL:m * W_FULL + w],
                            ft_sb[:, 2 * kp:2 * kp + 2, m * 128:(m + 1) * 128],
                            slabs[j][:, 2 * kp:2 * kp + 2, 0:w],
                            start=(kp == 0), stop=(kp == KPAIR - 1),
                            perf_mode=DROW)
                last.then_inc(sem_pe, 1)

        @block.scalar
        def _(scalar):
            # dummy exp: pulls the ACT_TABLE_LOAD (~1.3us) off the critical
            # path, overlapping the input DMA stream instead
            scalar.activation(out=scr[:, :], in_=small[:, 0, 0:2], func=EXP,
                              scale=ESCALE)
            # exp stream straight out of PSUM pairs, constant scale, bf16 out
            for n in range(NCHUNKS):
                b = n % NPSUM
                w = CW[n]
                pv = ps[b].rearrange("p (m w) -> p m w", m=2)
                scalar.wait_ge(sem_pe, n + 1)
                scalar.activation(
                    out=et[:, :, CO[n]:CO[n] + w],
                    in_=pv[:, :, 0:w],
                    func=EXP, scale=ESCALE).then_inc(sem_act, 1)

        @block.vector
        def _(vector):
            vcount = 0

            def v(instr):
                nonlocal vcount
                instr.then_inc(c_v, 1)
                vcount += 1
                return vcount

            # prefix sums over global columns [0,50)/[0,58) (host uses core 0's)
            vector.wait_ge(sem_act, 1)
            v(vector.tensor_reduce(out=small[:, :, 72:73], in_=et[:, :, 0:50],
                                   axis=AX, op=ADD))
            v(vector.tensor_reduce(out=small[:, :, 73:74], in_=et[:, :, 0:58],
                                   axis=AX, op=ADD))
            # per-chunk reductions right behind each exp: label-block sums and
            # camera-residue sums (chunks are 0 mod 8 wide -> fully aligned),
            # both 128-sample halves in one op
            for n in range(NCHUNKS):
                w = CW[n]
                nl = w // C
                co = CO[n] // C
                vector.wait_ge(sem_act, n + 1)
                chunk = et[:, :, CO[n]:CO[n] + w]
                v(vector.tensor_reduce(
                    out=bs[:, :, co:co + nl],
                    in_=chunk.rearrange("p m (l r) -> p m l r", r=C),
                    axis=AX, op=ADD))
                v(vector.tensor_reduce(
                    out=small[:, :, 8 * n:8 * n + 8],
                    in_=chunk.rearrange("p m (l r) -> p m r l", r=C),
                    axis=AX, op=ADD))

    return nc


_PROGRAM_CACHE: dict[str, bass.Bass] = {}


def _program() -> bass.Bass:
    if "nc" not in _PROGRAM_CACHE:
        _PROGRAM_CACHE["nc"] = _build_program()
    return _PROGRAM_CACHE["nc"]


def _make_in_maps(feats, centers, norms):
    f8 = ml_dtypes.float8_e4m3
    fn = feats / norms[:, None].astype(np.float32)     # unit rows
    fT_host = np.ascontiguousarray(fn.T)               # [2048, 256] f32
    fT8 = np.clip(fT_host * SF, -240.0, 240.0).astype(f8)
    fT8 = np.ascontiguousarray(fT8.reshape(KT, 128, N).transpose(1, 0, 2))
    cT8 = np.clip(np.ascontiguousarray(centers.T) * SC,
                  -240.0, 240.0).astype(f8)            # [2048, 32000] fp8

    in_maps = []
    for c in range(NCORES):
        shard = cT8[:, c * SHARD:(c + 1) * SHARD]        # [2048, 4000]
        sk = shard.reshape(KT, 128, SHARD)               # [16, 128, 4000]
        a = sk[:, :, 0:7 * W_FULL].reshape(KT, 128, 7, W_FULL)
        a = np.ascontiguousarray(a.transpose(2, 1, 0, 3))  # [7, 128, 16, 512]
        b7 = np.ascontiguousarray(
            sk[:, :, CO[7]:CO[8]].transpose(1, 0, 2))      # [128, 16, 256]
        b8 = np.ascontiguousarray(
            sk[:, :, CO[8]:].transpose(1, 0, 2))           # [128, 16, 160]
        in_maps.append({"cTa": a, "cTb7": b7, "cTb8": b8, "fT": fT8})
    return in_maps


def _host_tail(results, labels, camids, epoch, own):
    n = labels.shape[0]
    # SM_out [128, 2, SM_W]: sample i lives at [i % 128, i // 128, :]
    SM = [r["SM_out"].transpose(1, 0, 2).reshape(n, SM_W) for r in results]
    # per-chunk camera-residue sums (aligned: just sum over chunks and cores)
    S = np.zeros((n, C), np.float32)
    for sm in SM:
        S += sm[:, 0:8 * NCHUNKS].reshape(n, NCHUNKS, C).sum(axis=1)
    denom_intra = S[np.arange(n), camids]

    owner = (labels // LBL_SHARD).astype(np.int64)
    BS = np.stack([r["BS_out"].transpose(1, 0, 2).reshape(n, LBL_SHARD)
                   for r in results])
    B = BS[owner, np.arange(n), labels % LBL_SHARD]
    p50, p58 = SM[0][:, 72], SM[0][:, 73]
    hard = np.where(labels <= 6, p58 - B, p50)
    denom_inter = B + hard

    loss_i = own - np.log(denom_intra)
    loss_j = own - np.log(denom_inter)

    cam_sums = np.zeros(C, np.float32)
    cam_cnts = np.zeros(C, np.float32)
    np.add.at(cam_sums, camids, loss_i.astype(np.float32))
    np.add.at(cam_cnts, camids, 1.0)
    loss_intra = -np.sum(
        np.where(cam_cnts > 0, cam_sums / np.maximum(cam_cnts, 1.0), 0.0),
        dtype=np.float32)

    lbl_sums = np.zeros(L, np.float32)
    lbl_cnts = np.zeros(L, np.float32)
    np.add.at(lbl_sums, labels, loss_j.astype(np.float32))
    np.add.at(lbl_cnts, labels, 1.0)
    loss_inter = -np.sum(
        np.where(lbl_cnts > 0, lbl_sums / np.maximum(lbl_cnts, 1.0), 0.0),
        dtype=np.float32)

    if int(epoch) < 5:
        return np.float32(loss_intra)
    return np.stack([loss_intra, LAMDA * loss_inter]).astype(np.float32)


def kernel(feats, centers, labels, camids, epoch):
    feats = np.ascontiguousarray(np.asarray(feats, dtype=np.float32))
    centers = np.ascontiguousarray(np.asarray(centers, dtype=np.float32))
    labels = np.asarray(labels).astype(np.int64)
    camids = np.asarray(camids).astype(np.int64)

    norms = np.linalg.norm(feats.astype(np.float64), axis=1)
    own_idx = labels * C + camids
    own = np.einsum("ij,ij->i", feats.astype(np.float64),
                    centers[own_idx].astype(np.float64)) / (T * norms)

    in_maps = _make_in_maps(feats, centers, norms)
    res = run_bass_kernel_spmd(_program(), in_maps, list(range(NCORES))).results
    return _host_tail(res, labels, camids, epoch, own)
